# revision 12
# baseline (speedup 1.0000x reference)
"""Trainium2 Bass kernel for DifferentialMultiHeadAttentionWithHierarchicalMask.

Contract: kernel(**inputs) takes FULL unsharded numpy inputs (keys as in
setup_inputs) and returns (output, attn1, attn2) matching the reference.

Sharding: 8 cores; core c handles batch b=c//2 and heads [hs, hs+8) with
hs=(c%2)*8.  Everything is computed on-device; the host only reshapes /
transposes for layout, sums the two per-batch out-projection partials,
adds out_b, and casts bf16 attention outputs to fp32.
"""

import numpy as np
from contextlib import ExitStack
from ml_dtypes import bfloat16

import concourse.bacc as bacc
import concourse.tile as tile
from concourse import mybir
from concourse.bass_utils import run_bass_kernel_spmd
from concourse.masks import make_identity

B, S, D, H = 4, 1024, 1024, 16
DH = 64
HL = 8            # heads per core
CH = HL * DH      # 512 local channels
KT = D // 128     # 8 tiles over contraction dim
ST = S // 128     # 8 seq tiles
PT = CH // 128    # 4 partition tiles of local channels
GN_EPS = 1e-5
POST = 0.2        # 1 - LAMBDA_INIT
NCORES = 8

F32 = mybir.dt.float32
BF16 = mybir.dt.bfloat16
AF = mybir.ActivationFunctionType
ALU = mybir.AluOpType


def _build_program(lam: float, has_mask: bool, has_qkbias: bool, has_vbias: bool):
    nc = bacc.Bacc("TRN2", target_bir_lowering=False, debug=False, num_devices=NCORES)

    # ---- dram I/O (per core) ----
    xT_d = nc.dram_tensor("xT", [D, S], BF16, kind="ExternalInput")
    wq1_d = nc.dram_tensor("wq1T", [D, CH], BF16, kind="ExternalInput")
    wq2_d = nc.dram_tensor("wq2T", [D, CH], BF16, kind="ExternalInput")
    wk1_d = nc.dram_tensor("wk1T", [D, CH], BF16, kind="ExternalInput")
    wk2_d = nc.dram_tensor("wk2T", [D, CH], BF16, kind="ExternalInput")
    wv_d = nc.dram_tensor("wvT", [D, CH], BF16, kind="ExternalInput")
    wo_d = nc.dram_tensor("woT", [CH, D], BF16, kind="ExternalInput")
    ctb_d = nc.dram_tensor("ctb", [128, KT, S], BF16, kind="ExternalInput")
    gnw_d = nc.dram_tensor("gnw", [CH], F32, kind="ExternalInput")
    gnb_d = nc.dram_tensor("gnb", [CH], F32, kind="ExternalInput")
    if has_mask:
        mb_d = nc.dram_tensor("maskb", [S], F32, kind="ExternalInput")
    if has_qkbias:
        qb1_d = nc.dram_tensor("qb1", [CH], F32, kind="ExternalInput")
        qb2_d = nc.dram_tensor("qb2", [CH], F32, kind="ExternalInput")
        kb1_d = nc.dram_tensor("kb1", [CH], F32, kind="ExternalInput")
        kb2_d = nc.dram_tensor("kb2", [CH], F32, kind="ExternalInput")
    if has_vbias:
        vb_d = nc.dram_tensor("vb", [CH], F32, kind="ExternalInput")

    attn1_d = nc.dram_tensor("attn1", [HL, S, S], BF16, kind="ExternalOutput")
    attn2_d = nc.dram_tensor("attn2", [HL, S, S], BF16, kind="ExternalOutput")
    outp_d = nc.dram_tensor("outp", [S, D], F32, kind="ExternalOutput")

    with tile.TileContext(nc) as tc, ExitStack() as ctx:
        const = ctx.enter_context(tc.tile_pool(name="const", bufs=1))
        persist = ctx.enter_context(tc.tile_pool(name="persist", bufs=1))

        ident = const.tile([128, 128], BF16)
        make_identity(nc, ident)
        ones_sb = const.tile([128, 1], F32)
        nc.vector.memset(ones_sb, 1.0)
        eps_sb = const.tile([1, 1], F32)
        nc.vector.memset(eps_sb, GN_EPS)
        gnw_sb = const.tile([128, PT], F32)
        gnb_sb = const.tile([128, PT], F32)
        nc.sync.dma_start(out=gnw_sb, in_=gnw_d.ap().rearrange("(j p) -> p j", p=128))
        nc.sync.dma_start(out=gnb_sb, in_=gnb_d.ap().rearrange("(j p) -> p j", p=128))
        wo_sb = const.tile([128, PT, D], BF16)
        nc.sync.dma_start(out=wo_sb, in_=wo_d.ap().rearrange("(j p) e -> p j e", p=128))
        ctb_sb = const.tile([128, KT, S], BF16)
        nc.sync.dma_start(out=ctb_sb, in_=ctb_d.ap())
        if has_mask:
            mb_sb = const.tile([128, S], F32)
            nc.gpsimd.dma_start(out=mb_sb, in_=mb_d.ap().to_broadcast((128, S)))
        if has_qkbias:
            qkb_sb = {}
            for nm, dd in (("qb1", qb1_d), ("qb2", qb2_d), ("kb1", kb1_d), ("kb2", kb2_d)):
                t = const.tile([128, PT], F32, tag=f"bias_{nm}")
                nc.sync.dma_start(out=t, in_=dd.ap().rearrange("(j p) -> p j", p=128))
                qkb_sb[nm] = t
        if has_vbias:
            vbb_sb = const.tile([128, CH], F32)
            nc.gpsimd.dma_start(out=vbb_sb, in_=vb_d.ap().to_broadcast((128, CH)))

        # persistent across phases
        q1_sb = persist.tile([128, PT, S], BF16)
        q2_sb = persist.tile([128, PT, S], BF16)
        k1_sb = persist.tile([128, PT, S], BF16)
        k2_sb = persist.tile([128, PT, S], BF16)
        v_sb = persist.tile([128, ST, CH], BF16)
        ctxT_sb = persist.tile([128, PT, S], F32)   # [channel-part, (pair col), q]
        scaled_sb = persist.tile([128, PT, S], BF16)

        # ---------------- phase 1: projections ----------------
        with tc.tile_pool(name="ph1", bufs=1) as ph1, \
             tc.tile_pool(name="ph1ps", bufs=2, space="PSUM") as ph1ps, \
             tc.tile_pool(name="ph1psv", bufs=2, space="PSUM") as ph1psv:
            xT_sb = ph1.tile([128, KT, S], BF16)
            nc.sync.dma_start(out=xT_sb, in_=xT_d.ap().rearrange("(j p) s -> p j s", p=128))
            w_sb = {}
            for nm, dd in (("q1", wq1_d), ("q2", wq2_d), ("k1", wk1_d),
                           ("k2", wk2_d), ("v", wv_d)):
                t = ph1.tile([128, KT, CH], BF16, tag=f"w_{nm}")
                nc.sync.dma_start(out=t, in_=dd.ap().rearrange("(j p) c -> p j c", p=128))
                w_sb[nm] = t

            # q1/q2/k1/k2 in [channel-part, s] layout
            for nm, dst in (("q1", q1_sb), ("q2", q2_sb), ("k1", k1_sb), ("k2", k2_sb)):
                for i in range(PT):
                    pp = ph1ps.tile([128, S], F32, tag="pp")
                    for cc in range(2):
                        for j in range(KT):
                            nc.tensor.matmul(
                                pp[:, cc * 512:(cc + 1) * 512],
                                w_sb[nm][:, j, i * 128:(i + 1) * 128],
                                xT_sb[:, j, cc * 512:(cc + 1) * 512],
                                start=(j == 0), stop=(j == KT - 1))
                    if has_qkbias:
                        nc.vector.tensor_scalar_add(
                            out=dst[:, i, :], in0=pp, scalar1=qkb_sb["q" + nm[1] if nm[0] == "q" else "k" + nm[1]][:, i:i + 1])
                    else:
                        nc.vector.tensor_copy(dst[:, i, :], pp)

            # v in [s-part, channel] layout
            for st in range(ST):
                pv = ph1psv.tile([128, CH], F32, tag="pv")
                for j in range(KT):
                    nc.tensor.matmul(
                        pv[:, :],
                        xT_sb[:, j, st * 128:(st + 1) * 128],
                        w_sb["v"][:, j, :],
                        start=(j == 0), stop=(j == KT - 1))
                if has_vbias:
                    nc.vector.tensor_tensor(out=v_sb[:, st, :], in0=pv, in1=vbb_sb[:, :CH], op=ALU.add)
                else:
                    nc.vector.tensor_copy(v_sb[:, st, :], pv)

        # ---------------- phase 2: attention ----------------
        with tc.tile_pool(name="sc_ps", bufs=2, space="PSUM") as sc_ps, \
             tc.tile_pool(name="dt_ps", bufs=1, space="PSUM") as dt_ps, \
             tc.tile_pool(name="cx_ps", bufs=2, space="PSUM") as cx_ps, \
             tc.tile_pool(name="ew", bufs=4) as ew, \
             tc.tile_pool(name="aw", bufs=4) as aw, \
             tc.tile_pool(name="dw", bufs=2) as dw, \
             tc.tile_pool(name="cw", bufs=3) as cw, \
             tc.tile_pool(name="sw", bufs=3) as sw:
            for hp in range(4):
                for qg in range(2):
                    combs = []
                    for sub in range(2):
                        h = hp * 2 + sub
                        i = h // 2
                        base = (h % 2) * 64
                        comb = cw.tile([128, KT, 512], BF16, tag="comb")
                        combs.append(comb)
                        for qq in range(4):
                            qt = qg * 4 + qq
                            ps1 = sc_ps.tile([128, S], F32, tag="ps")
                            ps2 = sc_ps.tile([128, S], F32, tag="ps")
                            for ps, qsb, ksb in ((ps1, q1_sb, k1_sb), (ps2, q2_sb, k2_sb)):
                                for kc in range(2):
                                    nc.tensor.matmul(
                                        ps[:, kc * 512:(kc + 1) * 512],
                                        qsb[base:base + 64, i, qt * 128:(qt + 1) * 128],
                                        ksb[base:base + 64, i, kc * 512:(kc + 1) * 512],
                                        start=True, stop=True)
                            if has_mask:
                                nc.vector.tensor_tensor(out=ps1[:, :], in0=ps1, in1=mb_sb, op=ALU.add)
                                nc.vector.tensor_tensor(out=ps2[:, :], in0=ps2, in1=mb_sb, op=ALU.add)
                            sums = sw.tile([128, 2], F32, tag="sums")
                            e1 = ew.tile([128, S], BF16, tag="e")
                            e2 = ew.tile([128, S], BF16, tag="e")
                            nc.scalar.activation(out=e1, in_=ps1, func=AF.Exp,
                                                 scale=0.125, accum_out=sums[:, 0:1])
                            nc.scalar.activation(out=e2, in_=ps2, func=AF.Exp,
                                                 scale=0.125, accum_out=sums[:, 1:2])
                            rr = sw.tile([128, 2], F32, tag="rr")
                            nc.vector.reciprocal(out=rr, in_=sums)
                            a1 = aw.tile([128, S], BF16, tag="a")
                            a2 = aw.tile([128, S], BF16, tag="a")
                            nc.vector.tensor_scalar_mul(out=a1, in0=e1, scalar1=rr[:, 0:1])
                            nc.vector.tensor_scalar_mul(out=a2, in0=e2, scalar1=rr[:, 1:2])
                            nc.sync.dma_start(out=attn1_d[h, qt * 128:(qt + 1) * 128, :], in_=a1)
                            nc.sync.dma_start(out=attn2_d[h, qt * 128:(qt + 1) * 128, :], in_=a2)
                            dtile = dw.tile([128, S], BF16, tag="d")
                            nc.vector.scalar_tensor_tensor(
                                out=dtile, in0=a2, scalar=-lam, in1=a1,
                                op0=ALU.mult, op1=ALU.add)
                            dtp = dt_ps.tile([128, KT, 128], BF16, tag="dtp")
                            for j in range(KT):
                                nc.tensor.transpose(dtp[:, j, :], dtile[:, j * 128:(j + 1) * 128], ident)
                            nc.vector.tensor_tensor(
                                out=comb[:, :, qq * 128:(qq + 1) * 128],
                                in0=dtp,
                                in1=ctb_sb[:, :, qt * 128:(qt + 1) * 128],
                                op=ALU.mult)
                    # ctx for this (pair, qgroup): two heads packed into col halves
                    cps = cx_ps.tile([128, 512], F32, tag="cps")
                    for sub in range(2):
                        h = hp * 2 + sub
                        for j in range(KT):
                            nc.tensor.matmul(
                                cps[sub * 64:(sub + 1) * 64, :],
                                v_sb[:, j, h * 64:(h + 1) * 64],
                                combs[sub][:, j, :],
                                start=(j == 0), stop=(j == KT - 1),
                                tile_position=(0, sub * 64))
                    nc.vector.tensor_copy(ctxT_sb[:, hp, qg * 512:(qg + 1) * 512], cps)

        # ---------------- phase 3: GroupNorm ----------------
        with tc.tile_pool(name="gn", bufs=8) as gn, \
             tc.tile_pool(name="gnd", bufs=4, space="DRAM") as gnd, \
             tc.tile_pool(name="gn_ps", bufs=2, space="PSUM") as gn_ps:
            for hp in range(4):
                ctxc = ctxT_sb[:, hp, :]
                stt = gn.tile([128, 2, 6], F32, tag="stt")
                nc.vector.bn_stats(out=stt[:, 0, :], in_=ctxc[:, 0:512])
                nc.vector.bn_stats(out=stt[:, 1, :], in_=ctxc[:, 512:1024])
                mv = gn.tile([128, 2], F32, tag="mv")
                nc.vector.bn_aggr(out=mv, in_=stt)
                # msq: col0 = mean_p, col1 = mean_p^2 + var_p
                msq = gn.tile([128, 2], F32, tag="msq")
                nc.vector.tensor_copy(msq[:, 0:1], mv[:, 0:1])
                nc.vector.scalar_tensor_tensor(
                    out=msq[:, 1:2], in0=mv[:, 0:1], scalar=mv[:, 0:1],
                    in1=mv[:, 1:2], op0=ALU.mult, op1=ALU.add)
                mbrb = gn.tile([128, 2], F32, tag="mbrb")  # col0=mean col1=rstd
                for sub in range(2):
                    bb = sub * 64
                    sp = gn_ps.tile([1, 2], F32, tag="sp")
                    nc.tensor.matmul(sp, ones_sb[bb:bb + 64, :], msq[bb:bb + 64, :],
                                     start=True, stop=True)
                    sc = gn.tile([1, 2], F32, tag="sc")
                    nc.vector.tensor_scalar_mul(out=sc, in0=sp, scalar1=1.0 / 64.0)
                    nvar = gn.tile([1, 1], F32, tag="nvar")
                    # nvar = mean^2 - E[x^2] = -var
                    nc.vector.scalar_tensor_tensor(
                        out=nvar, in0=sc[:, 0:1], scalar=sc[:, 0:1],
                        in1=sc[:, 1:2], op0=ALU.mult, op1=ALU.subtract)
                    lv = gn.tile([1, 1], F32, tag="lv")
                    nc.scalar.activation(out=lv, in_=nvar, func=AF.Ln,
                                         scale=-1.0, bias=eps_sb)
                    scm = gn.tile([1, 2], F32, tag="scm")
                    nc.vector.tensor_copy(scm[:, 0:1], sc[:, 0:1])
                    # rstd = exp(-0.5*ln(var+eps))
                    nc.scalar.activation(out=scm[:, 1:2], in_=lv, func=AF.Exp, scale=-0.5)
                    scr = gnd.tile([1, 2], F32, tag="scr")
                    nc.gpsimd.dma_start(out=scr[:, :], in_=scm[:, :])
                    nc.gpsimd.dma_start(out=mbrb[bb:bb + 64, :],
                                        in_=scr[:, :].to_broadcast((64, 2)))
                acol = gn.tile([128, 1], F32, tag="acol")
                nc.vector.tensor_tensor(out=acol, in0=mbrb[:, 1:2],
                                        in1=gnw_sb[:, hp:hp + 1], op=ALU.mult)
                bneg = gn.tile([128, 1], F32, tag="bneg")
                nc.vector.scalar_tensor_tensor(
                    out=bneg, in0=acol, scalar=mbrb[:, 0:1], in1=gnb_sb[:, hp:hp + 1],
                    op0=ALU.mult, op1=ALU.subtract)
                nc.vector.tensor_scalar(
                    out=scaled_sb[:, hp, :], in0=ctxc, scalar1=acol, scalar2=bneg,
                    op0=ALU.mult, op1=ALU.subtract)

        # ---------------- phase 4: out projection ----------------
        with tc.tile_pool(name="op", bufs=2) as op_pool, \
             tc.tile_pool(name="op_ps", bufs=2, space="PSUM") as op_ps:
            for st in range(ST):
                op = op_ps.tile([128, D], F32, tag="op")
                for cc in range(2):
                    for hp in range(PT):
                        nc.tensor.matmul(
                            op[:, cc * 512:(cc + 1) * 512],
                            scaled_sb[:, hp, st * 128:(st + 1) * 128],
                            wo_sb[:, hp, cc * 512:(cc + 1) * 512],
                            start=(hp == 0), stop=(hp == PT - 1))
                oc = op_pool.tile([128, D], F32, tag="oc")
                nc.vector.tensor_copy(oc, op)
                nc.sync.dma_start(out=outp_d[st * 128:(st + 1) * 128, :], in_=oc)

    nc.compile()
    return nc


_prog_cache = {}


def _get_program(lam, has_mask, has_qkbias, has_vbias):
    key = (round(float(lam), 8), has_mask, has_qkbias, has_vbias)
    if key not in _prog_cache:
        _prog_cache[key] = _build_program(float(lam), has_mask, has_qkbias, has_vbias)
    return _prog_cache[key]


def _prepare_in_maps(x_norm, hierarchical_mask_C, padding_mask,
                     wq_w, wq_b, wk_w, wk_b, wv_w, wv_b, out_w,
                     gn_w, gn_b, has_mask, has_qkbias, has_vbias):
    in_maps = []
    gnw_s = (gn_w * POST).astype(np.float32)
    gnb_s = (gn_b * POST).astype(np.float32)
    wq_T = np.ascontiguousarray(wq_w.T).astype(bfloat16)   # [D, 2D]
    wk_T = np.ascontiguousarray(wk_w.T).astype(bfloat16)
    wv_T = np.ascontiguousarray(wv_w.T).astype(bfloat16)   # [D, D]
    wo_T = np.ascontiguousarray(out_w.T).astype(bfloat16)  # [D, D]
    for c in range(NCORES):
        b = c // 2
        hs = (c % 2) * HL
        lo, hi = hs * DH, hs * DH + CH
        m = {
            "xT": np.ascontiguousarray(x_norm[b].T).astype(bfloat16),
            "wq1T": np.ascontiguousarray(wq_T[:, lo:hi]),
            "wq2T": np.ascontiguousarray(wq_T[:, D + lo:D + hi]),
            "wk1T": np.ascontiguousarray(wk_T[:, lo:hi]),
            "wk2T": np.ascontiguousarray(wk_T[:, D + lo:D + hi]),
            "wvT": np.ascontiguousarray(wv_T[:, lo:hi]),
            "woT": np.ascontiguousarray(wo_T[lo:hi, :]),
            # ctb[p, j, q] = C[b,0][q, j*128+p]
            "ctb": np.ascontiguousarray(
                hierarchical_mask_C[b, 0].T.reshape(KT, 128, S).transpose(1, 0, 2)
            ).astype(bfloat16),
            "gnw": np.ascontiguousarray(gnw_s[lo:hi]),
            "gnb": np.ascontiguousarray(gnb_s[lo:hi]),
        }
        if has_mask:
            m["maskb"] = np.where(padding_mask[b] == 0, -8e9, 0.0).astype(np.float32)
        if has_qkbias:
            m["qb1"] = np.ascontiguousarray(wq_b[lo:hi]).astype(np.float32)
            m["qb2"] = np.ascontiguousarray(wq_b[D + lo:D + hi]).astype(np.float32)
            m["kb1"] = np.ascontiguousarray(wk_b[lo:hi]).astype(np.float32)
            m["kb2"] = np.ascontiguousarray(wk_b[D + lo:D + hi]).astype(np.float32)
        if has_vbias:
            m["vb"] = np.ascontiguousarray(wv_b[lo:hi]).astype(np.float32)
        in_maps.append(m)
    return in_maps


def kernel(x_norm, learned_lambda, hierarchical_mask_C, padding_mask,
           wq_w, wq_b, wk_w, wk_b, wv_w, wv_b, out_w, out_b, gn_w, gn_b,
           _return_raw=False, _trace=False):
    x_norm = np.asarray(x_norm, dtype=np.float32)
    learned_lambda = np.asarray(learned_lambda, dtype=np.float32)
    hierarchical_mask_C = np.asarray(hierarchical_mask_C, dtype=np.float32)
    padding_mask = np.asarray(padding_mask)
    wq_w = np.asarray(wq_w, dtype=np.float32); wq_b = np.asarray(wq_b, dtype=np.float32)
    wk_w = np.asarray(wk_w, dtype=np.float32); wk_b = np.asarray(wk_b, dtype=np.float32)
    wv_w = np.asarray(wv_w, dtype=np.float32); wv_b = np.asarray(wv_b, dtype=np.float32)
    out_w = np.asarray(out_w, dtype=np.float32); out_b = np.asarray(out_b, dtype=np.float32)
    gn_w = np.asarray(gn_w, dtype=np.float32); gn_b = np.asarray(gn_b, dtype=np.float32)

    lam = float(learned_lambda.reshape(-1)[0])
    has_mask = bool(np.any(padding_mask == 0))
    has_qkbias = bool(np.any(wq_b) or np.any(wk_b))
    has_vbias = bool(np.any(wv_b))

    nc = _get_program(lam, has_mask, has_qkbias, has_vbias)
    in_maps = _prepare_in_maps(x_norm, hierarchical_mask_C, padding_mask,
                               wq_w, wq_b, wk_w, wk_b, wv_w, wv_b, out_w,
                               gn_w, gn_b, has_mask, has_qkbias, has_vbias)

    res = run_bass_kernel_spmd(nc, in_maps, core_ids=list(range(NCORES)))

    attn1 = np.empty((B, H, S, S), dtype=np.float32)
    attn2 = np.empty((B, H, S, S), dtype=np.float32)
    output = np.empty((B, S, D), dtype=np.float32)
    for b in range(B):
        r0 = res.results[2 * b]
        r1 = res.results[2 * b + 1]
        attn1[b, :HL] = r0["attn1"].astype(np.float32)
        attn1[b, HL:] = r1["attn1"].astype(np.float32)
        attn2[b, :HL] = r0["attn2"].astype(np.float32)
        attn2[b, HL:] = r1["attn2"].astype(np.float32)
        output[b] = r0["outp"] + r1["outp"] + out_b[None, :]
    if _return_raw:
        return (output, attn1, attn2), res
    return output, attn1, attn2


def benchmark(inputs, iters=24, warmup=4):
    """Measure steady-state device execution time of the compiled kernel via
    async-pipelined repeated execution on device-resident inputs (the NTFF
    profiling hook is unavailable under this axon client, so we use the
    dispatch-slope method).  Returns (ns_per_iter, details)."""
    import time
    import jax
    from jax.sharding import Mesh, PartitionSpec
    from jax.experimental.shard_map import shard_map
    from concourse import bass2jax, mybir as _mb

    x = {k: np.asarray(v) for k, v in inputs.items()}
    lam = float(np.asarray(x["learned_lambda"]).reshape(-1)[0])
    has_mask = bool(np.any(np.asarray(x["padding_mask"]) == 0))
    has_qkbias = bool(np.any(x["wq_b"]) or np.any(x["wk_b"]))
    has_vbias = bool(np.any(x["wv_b"]))
    nc = _get_program(lam, has_mask, has_qkbias, has_vbias)
    in_maps = _prepare_in_maps(
        np.asarray(x["x_norm"], np.float32), np.asarray(x["hierarchical_mask_C"], np.float32),
        np.asarray(x["padding_mask"]),
        np.asarray(x["wq_w"], np.float32), np.asarray(x["wq_b"], np.float32),
        np.asarray(x["wk_w"], np.float32), np.asarray(x["wk_b"], np.float32),
        np.asarray(x["wv_w"], np.float32), np.asarray(x["wv_b"], np.float32),
        np.asarray(x["out_w"], np.float32),
        np.asarray(x["gn_w"], np.float32), np.asarray(x["gn_b"], np.float32),
        has_mask, has_qkbias, has_vbias)

    bass2jax.install_neuronx_cc_hook()
    partition_name = nc.partition_id_tensor.name if nc.partition_id_tensor else None
    in_names, out_names, out_avals, zero_outs = [], [], [], []
    for alloc in nc.m.functions[0].allocations:
        if not isinstance(alloc, _mb.MemoryLocationSet):
            continue
        name = alloc.memorylocations[0].name
        if alloc.kind == "ExternalInput":
            if name != partition_name:
                in_names.append(name)
        elif alloc.kind == "ExternalOutput":
            out_names.append(name)
            shape = tuple(alloc.tensor_shape)
            dtype = _mb.dt.np(alloc.dtype)
            out_avals.append(jax.core.ShapedArray(shape, dtype))
            zero_outs.append(np.zeros(shape, dtype))
    n_params = len(in_names)
    all_names = in_names + out_names
    if partition_name is not None:
        all_names = all_names + [partition_name]

    def _body(*args):
        operands = list(args)
        if partition_name is not None:
            operands.append(bass2jax.partition_id_tensor())
        outs = bass2jax._bass_exec_p.bind(
            *operands, out_avals=tuple(out_avals), in_names=tuple(all_names),
            out_names=tuple(out_names), lowering_input_output_aliases=(),
            sim_require_finite=True, sim_require_nnan=True, nc=nc)
        return tuple(outs)

    devices = jax.devices()[:NCORES]
    mesh = Mesh(np.asarray(devices), ("core",))
    n_outs = len(out_names)
    nin = n_params + n_outs
    donate = tuple(range(n_params, nin))
    fn = jax.jit(shard_map(_body, mesh=mesh,
                           in_specs=(PartitionSpec("core"),) * nin,
                           out_specs=(PartitionSpec("core"),) * n_outs,
                           check_rep=False),
                 donate_argnums=donate, keep_unused=True)
    concat_in = [np.concatenate([np.asarray(in_maps[c][nm]) for c in range(NCORES)], axis=0)
                 for nm in in_names]
    from jax.sharding import NamedSharding
    sh = NamedSharding(mesh, PartitionSpec("core"))
    dev_in = [jax.device_put(a, sh) for a in concat_in]
    zero_shapes = [(NCORES * z.shape[0], *z.shape[1:]) for z in zero_outs]
    zero_dtypes = [z.dtype for z in zero_outs]

    import jax.numpy as jnp
    mkzero = jax.jit(lambda: tuple(
        jnp.zeros(s, d) for s, d in zip(zero_shapes, zero_dtypes)),
        out_shardings=tuple(sh for _ in zero_shapes))

    def fresh_zeros():
        return [jax.device_put(z, sh) for z in jax.block_until_ready(mkzero())]

    # warmup + compile
    for _ in range(max(1, warmup)):
        out = fn(*dev_in, *fresh_zeros())
    jax.block_until_ready(out)

    def run_pipelined(n):
        zsets = [fresh_zeros() for _ in range(n)]
        t0 = time.perf_counter()
        last = None
        for k in range(n):
            last = fn(*dev_in, *zsets[k])
        jax.block_until_ready(last)
        return time.perf_counter() - t0

    n1, n2 = max(2, iters // 4), iters
    t_small = min(run_pipelined(n1) for _ in range(2))
    t_big = min(run_pipelined(n2) for _ in range(2))
    per_iter = (t_big - t_small) / (n2 - n1)

    # serial (dispatch-inclusive) single-call time for reference
    serial = []
    for _ in range(4):
        z = fresh_zeros()
        t0 = time.perf_counter()
        jax.block_until_ready(fn(*dev_in, *z))
        serial.append(time.perf_counter() - t0)
    details = dict(t_small=t_small, t_big=t_big, n1=n1, n2=n2,
                   serial_ms=[round(s * 1e3, 3) for s in serial])
    return per_iter * 1e9, details
